# revision 64
# baseline (speedup 1.0000x reference)
"""Trainium2 Bass kernel for nn_Attention_13829794693777.

Multi-head attention (8 heads, head_dim 48) + LePE depthwise 3x3 conv on v.
Sharding: tensor-parallel over heads -- one head per NeuronCore (8 cores).

v3 design: scores matmuls run in fp8e4m3 DoubleRow perf mode (0.5
PE-cycles/row, two 24-feature half-contractions fused per pass), halving
the dominant PE cost. q+k are projected through one packed 96-wide bf16
stationary, bias-applied + fp8-quantized in one ACT pass into a 4-chunk
staging tile, then partition-reshuffled [96,4x512]->[24,4,(4,512)] by four
SBUF->SBUF DMAs per 4-chunk group (engines cannot move data across
partitions; small DMAs can). exp uses a constant shift S0 (cancels in
softmax; keeps ranges in bf16/e4m3 bounds) and is split between ACT (real
Exp, scale=ln2/8, bias=-S0 -> bf16) and DVE (Schraudolph int16 bitcast,
x16 + C2). PV stays bf16. LePE runs fully on PE as 9 shifted diag-matmul
taps + ones-row bias matmul per query tile (GPSIMD cannot touch PSUM and
rejects AP-scalar ops, so it is unused). Each chunk's pvb accumulation
group opens with a full-height 128-partition zeroing matmul (a 64-row
start matmul would leave partitions 64-127 with stale has_written bits).
The epilogue is an ACT PSUM->SBUF copy, DVE reciprocal + affine
(attn*rec + lepe). Emission software-pipelines across chunks (previous
chunk's PV tail + epilogue interleave with the next chunk's scores) and
interleaves batch-1 projection and all v_aug into phase 1 to shorten the
startup. Exp engine-time is the binding resource: ~152us across ACT+DVE
(GPSIMD cannot read PSUM, so only these two engines can consume scores).
"""

import numpy as np
import ml_dtypes

NUM_HEADS = 8
DIM = 384
HD = 48
B = 2
N = 4096
SEQ = B * N          # 8192
IMG = 64             # H = W = 64
PADW = IMG + 2       # 66
PADN = PADW * PADW   # 4356
SCALE = HD ** -0.5
NCHUNK = SEQ // 512  # 16 query chunks of 512
KT_PER_B = N // 128  # 32 k-tiles per batch

LN2 = float(np.log(2.0))
S0 = 5.5                       # exp shift, cancels in softmax
QS = SCALE * 8.0 / LN2         # baked into wq/bq: s8 = nat_score * 8/ln2
ACT_SCALE = LN2 / 8.0          # ACT Exp: exp(s8*ACT_SCALE - S0)
C2EFF = 16250.5 - S0 * 128.0 / LN2   # DVE schraudolph: int16(s8*16 + C2EFF)

# exp engine pattern per chunk: batches of 2 k-tiles.
# 'A' = ACT real exp, 'D' = DVE Schraudolph. 10 chunks use 9A/7D and 6 use
# 8A/8D so the ACT/DVE exp loads balance against their fixed work.
EXP_PATTERNS = [
    ['A', 'A', 'D', 'A', 'D', 'A', 'D', 'A',
     'D', 'A', 'D', 'A', 'D', 'A', 'D', 'A'],
    ['A', 'D', 'D', 'A', 'D', 'A', 'D', 'A',
     'D', 'A', 'D', 'A', 'D', 'A', 'D', 'A'],
]
NINE_A = set(range(16))
EXP_BW = 2
PV_LAG = 8
PE_TAPS = [0, 4, 8]
LEPE_STEP = 6

_CACHE = {}


def _build_module():
    """Build (once) the Bacc module shared by all 8 cores."""
    import concourse.bacc as bacc
    import concourse.mybir as mybir
    import concourse.tile as tile
    from contextlib import ExitStack

    dt = mybir.dt
    AF = mybir.ActivationFunctionType
    ALU = mybir.AluOpType
    PM = mybir.MatmulPerfMode

    nc = bacc.Bacc("TRN2", target_bir_lowering=False, debug=False, num_devices=8)

    # ---- DRAM parameters -------------------------------------------------
    xT_d = nc.dram_tensor("xT", [3, 128, SEQ], dt.bfloat16, kind="ExternalInput").ap()
    # all weights/constants packed into one [128, 1024]-bf16 blob (single
    # DMA: the 9 small DMAs each cost 625ns on the serial HWDGE device and
    # delayed the x stream). Layout (bf16 cols): wqk 0:288, wv 288:432,
    # dwt 432:864, lb48 864:912, idb 912:976, then 3 fp32 scalars
    # (bqk, bv, eb) as raw bytes in cols 976:982.
    blob_d = nc.dram_tensor("blob", [128, 1024], dt.bfloat16, kind="ExternalInput").ap()
    out_d = nc.dram_tensor("out", [64, 128, HD], dt.float32, kind="ExternalOutput").ap()
    out_v = out_d.rearrange("t p c -> p t c")
    if _CACHE.get("debug"):
        dbg_va_d = nc.dram_tensor("dbg_va", [128, 64 * 49], dt.bfloat16,
                                  kind="ExternalOutput").ap()
        dbg_vp_d = nc.dram_tensor("dbg_vp", [128, PADN], dt.bfloat16,
                                  kind="ExternalOutput").ap()
        dbg_tmp_d = nc.dram_tensor("dbg_tmp", [128, 388], dt.float32,
                                   kind="ExternalOutput").ap()

    with tile.TileContext(nc) as tc, ExitStack() as ctx:
        per = ctx.enter_context(tc.tile_pool(name="per", bufs=1))
        rot = ctx.enter_context(tc.tile_pool(name="rot", bufs=3))
        stg_p = ctx.enter_context(tc.tile_pool(name="stgp", bufs=3))
        ptp = ctx.enter_context(tc.tile_pool(name="ptp", bufs=11))

        # ---- persistent SBUF tensors ------------------------------------
        xs_all = per.tile([128, 3 * SEQ], dt.bfloat16, name="xall", tag="xall")
        xs3 = xs_all[:].rearrange("p (c n) -> p c n", c=3)
        # DoubleRow-layout q/k: per chunk [24, 4, 512] fp8; t=0,1 are the two
        # 24-feature halves of q (QS-scaled), t=2,3 of k.
        qkdr = per.tile([128, NCHUNK * 2048], dt.float8e4, name="qkdr", tag="qkdr")
        qkdr4 = qkdr[:].rearrange("p (n t c) -> p n t c", t=4, c=512)
        v_aug = per.tile([128, 64 * 49], dt.bfloat16, name="vaug", tag="vaug")
        v_aug3 = v_aug[:].rearrange("p (t c) -> p t c", c=49)
        vT_pad = per.tile([128, PADN], dt.bfloat16, name="vpad", tag="vpad")
        vp3 = vT_pad[:].rearrange("p (a b) -> p a b", b=PADW)

        blob = per.tile([128, 1024], dt.bfloat16, name="blob", tag="blob")
        wqk_sb = [blob[:, 96 * c:96 * c + 96] for c in range(3)]
        wv_sb = [blob[:, 288 + HD * c:288 + HD * c + HD] for c in range(3)]
        dw_sb = [blob[:, 432 + HD * i:432 + HD * i + HD] for i in range(9)]
        lb_sb = blob[:, 864:912]
        idb_sb = blob[:, 912:976]
        f32v = blob[:, 976:982].bitcast(dt.float32)
        bqk_sb = f32v[:, 0:1]
        bv_sb = f32v[:, 1:2]
        eb_sb = f32v[:, 2:3]
        one_sb = per.tile([128, 128], dt.bfloat16, name="ones", tag="ones")
        zrow_sb = per.tile([128, 128], dt.bfloat16, name="zrow", tag="zrow")

        # ---- input DMAs: one blob DMA + x in 5 all-c pieces -------------
        # (merging the three c-tensors per piece costs the same DMA-device
        # time but 1/3 the serial HWDGE issue overhead)
        nc.scalar.dma_start(blob[:], blob_d)
        xTv = xT_d.rearrange("c p n -> p c n")
        nc.sync.dma_start(xs3[:, :, 0:512], xTv[:, :, 0:512])
        nc.sync.dma_start(xs3[:, :, 512:2048], xTv[:, :, 512:2048])
        for j in range(1, 4):
            nc.sync.dma_start(xs3[:, :, j * 2048:(j + 1) * 2048],
                              xTv[:, :, j * 2048:(j + 1) * 2048])
        nc.vector.memset(one_sb[0:1, :], 1.0)
        nc.vector.memset(zrow_sb[0:1, :], 0.0)

        # zero the padded image (borders must be 0)
        nc.vector.memset(vT_pad[:], 0.0)
        nc.vector.memset(v_aug3[:, :, 48:49], 1.0)

        taps = [(dr, dc) for dr in (-1, 0, 1) for dc in (-1, 0, 1)]

        # ---- projection chunk emitter -----------------------------------
        # ps2 (v_aug transposes) is used only during phase 1b; allocated
        # before psA so the LIFO release order works out.
        ps2_ctx = ExitStack()
        ps2 = ps2_ctx.enter_context(tc.tile_pool(name="ps2", bufs=2, space="PSUM"))
        psA_ctx = ExitStack()
        psA = psA_ctx.enter_context(tc.tile_pool(name="psA", bufs=3, space="PSUM"))
        stg_groups = {}

        def emit_proj(n):
            rhs = [xs3[:, c, 512 * n:512 * n + 512] for c in range(3)]
            b = n // 8
            rb = 64 * b
            pvv = psA.tile([128, 512], dt.float32, name="pvv", tag="pvv")
            for c in range(3):
                nc.tensor.matmul(pvv[rb:rb + HD, :], wv_sb[c], rhs[c],
                                 start=(c == 0), stop=(c == 2),
                                 tile_position=(0, rb))
            # scatter the 512 pixels (8 image rows) into the padded image
            r0 = 8 * (n % 8)
            dest = vp3[rb:rb + HD, 1 + r0:1 + r0 + 8, 1:65]
            nc.vector.tensor_scalar(dest, pvv[rb:rb + HD, :], bv_sb[rb:rb + HD, 0:1],
                                    None, op0=ALU.add)
            # packed q+k projection: one 96-wide stationary per c-tile
            pqk = psA.tile([128, 512], dt.float32, name="pqk", tag="pqk")
            for c in range(3):
                nc.tensor.matmul(pqk[0:96, :], wqk_sb[c], rhs[c],
                                 start=(c == 0), stop=(c == 2))
            g = n // 8
            if n % 8 == 0:
                stg_groups[g] = stg_p.tile([128, 4096], dt.float8e4,
                                           name="stg", tag="stg")
            stg = stg_groups[g]
            nc.scalar.activation(stg[0:96, 512 * (n % 8):512 * (n % 8) + 512],
                                 pqk[0:96, :], AF.Identity,
                                 bias=bqk_sb[0:96, 0:1])
            if n % 8 == 7:
                # partition reshuffle [96, 8x512] -> [24, 8, (4,512)]
                # (DoubleRow layout), one DMA per 24-feature quarter
                stg = stg_groups.pop(g)
                for t in range(4):
                    nc.sync.dma_start(qkdr4[0:24, 8 * g:8 * g + 8, t, :],
                                      stg[24 * t:24 * t + 24, :]
                                      .rearrange("p (n c) -> p n c", c=512))

        def emit_vaug(g):
            tq = ps2.tile([128, 192], dt.bfloat16, name="tq", tag="tq")
            for j in range(4):
                t = 4 * g + j
                b = t // 32
                rb = 64 * b
                tt = t % 32
                for h in range(2):
                    nc.tensor.matmul(tq[64 * h:64 * h + 64, 48 * j:48 * j + 48],
                                     vp3[rb:rb + HD, 1 + 2 * tt + h, 1:65],
                                     idb_sb[rb:rb + HD, 0:HD],
                                     is_transpose=True, tile_position=(rb, 64 * h))
            tq3 = tq[:].rearrange("p (t c) -> p t c", c=48)
            if g % 3 != 2:
                nc.scalar.activation(v_aug3[:, 4 * g:4 * g + 4, 0:48], tq3[:],
                                     AF.Copy)
            else:
                nc.vector.tensor_copy(v_aug3[:, 4 * g:4 * g + 4, 0:48], tq3[:])

        # ---- phase 1a: projection chunks 0-7 (batch 0) + v_aug 0-7 ------
        # v_aug group g needs image rows 8g..8g+8 = proj chunks g, g+1
        for n in range(8):
            emit_proj(n)
            if n >= 1:
                emit_vaug(n - 1)
        emit_vaug(7)

        # ---- phase 1b: projection 8-15 + v_aug groups 8-15 --------------
        # batch-1 group 8+i needs projection chunks 8+i and 9+i scattered
        for n in range(8, 16):
            emit_proj(n)
            if n >= 9:
                emit_vaug(n - 1)
        emit_vaug(15)
        psA_ctx.close()
        ps2_ctx.close()

        # ---- phase 2: main attention loop -------------------------------
        stp = ctx.enter_context(tc.tile_pool(name="stp", bufs=3, space="PSUM"))
        psv = ctx.enter_context(tc.tile_pool(name="psv", bufs=2, space="PSUM"))

        nb = KT_PER_B // EXP_BW
        batches = [(EXP_BW * i, EXP_BW * i + EXP_BW) for i in range(nb)]

        def emit_epi_copy(cc, pvb):
            # DVE: one PSUM->SBUF copy of attn (cols 0:196) + lepe (196:388).
            tmp = rot.tile([128, 388], dt.float32, name="tmp", tag="tmp")
            nc.vector.tensor_copy(tmp[:], pvb[:, 0:388])
            return tmp

        def emit_epi_rest(cc, pvb, tmp):
            # DVE: reciprocal of the denominators + affine attn*rec + lepe.
            rec = rot.tile([128, 4], dt.float32, name="rec", tag="rec")
            ot = rot.tile([128, 192], dt.float32, name="ot", tag="ot")
            tmp3 = tmp[:, 0:196].rearrange("p (t c) -> p t c", c=49)
            nc.vector.reciprocal(rec[:], tmp3[:, :, 48:49])
            for qs in range(4):
                nc.vector.scalar_tensor_tensor(
                    ot[:, qs * 48:(qs + 1) * 48],
                    tmp[:, qs * 49:qs * 49 + 48],
                    rec[:, qs:qs + 1],
                    tmp[:, 196 + 48 * qs:196 + 48 * qs + 48],
                    op0=ALU.mult, op1=ALU.add)
            nc.sync.dma_start(out_v[:, 4 * cc:4 * cc + 4, :],
                              ot[:].rearrange("p (t c) -> p t c", c=HD))
            if _CACHE.get("debug") and cc == 0:
                nc.sync.dma_start(dbg_tmp_d, tmp[:])

        prev = None
        for cc in range(NCHUNK):
            bc = cc // 8
            rb = 64 * bc
            pattern = EXP_PATTERNS[0 if cc in NINE_A else 1]
            pvb = psv.tile([128, 512], dt.float32, name="pvb", tag="pvb")
            pv3 = pvb[:, 0:196].rearrange("p (t c) -> p t c", c=49)

            def emit_lepe_taps(qs):
                # all 9 shifted diag-matmul taps + 1 ones-row bias matmul,
                # accumulated straight into the chunk's PSUM epilogue region
                tt = (4 * cc + qs) % 32
                for h in range(2):
                    dst = pvb[64 * h:64 * h + 64, 196 + 48 * qs:196 + 48 * qs + 48]
                    for ti, (dr, dc) in enumerate(taps):
                        nc.tensor.matmul(dst,
                                         vp3[rb:rb + HD, 1 + 2 * tt + h + dr,
                                             1 + dc:1 + dc + IMG],
                                         dw_sb[ti][rb:rb + HD, :],
                                         start=False, stop=False,
                                         tile_position=(rb, 64 * h),
                                         skip_group_check=True)
                    nc.tensor.matmul(dst, one_sb[0:1, 0:64], lb_sb[0:1, :],
                                     start=False, stop=False,
                                     tile_position=(0, 64 * h),
                                     skip_group_check=True)

            pt_of_batch = {}
            bi_box = [0]  # next batch whose scores are fully issued
            st_of_batch = {}

            def emit_exp(bidx):
                w = 512 * EXP_BW
                st = st_of_batch.pop(bidx)
                pt = ptp.tile([128, 512 * EXP_BW], dt.bfloat16, name="pt", tag="pt")
                if pattern[bidx] == 'A':
                    nc.scalar.activation(pt[:, 0:w], st[:, 0:w],
                                         AF.Exp, scale=ACT_SCALE,
                                         bias=eb_sb[:, 0:1])
                else:
                    nc.vector.tensor_scalar(
                        pt[:, 0:w].bitcast(mybir.dt.int16),
                        st[:, 0:w], 16.0, C2EFF, op0=ALU.mult, op1=ALU.add)
                pt_of_batch[bidx] = pt

            def make_pv(pvb_, pt_map_, bc_):
                def emit_pv(kt):
                    bidx = kt // EXP_BW
                    a, _ = batches[bidx]
                    pt = pt_map_[bidx]
                    for qb in range(4):
                        nc.tensor.matmul(pvb_[:, 49 * qb:49 * qb + 49],
                                         pt[:, 512 * (kt - a) + 128 * qb:
                                             512 * (kt - a) + 128 * qb + 128],
                                         v_aug3[:, bc_ * 32 + kt, 0:49],
                                         start=False,
                                         stop=(kt == KT_PER_B - 1 and qb == 3),
                                         tile_position=(0, 0),
                                         skip_group_check=True)
                return emit_pv

            emit_pv = make_pv(pvb, pt_of_batch, bc)

            # 32 steps; the previous chunk's PV tail interleaves with this
            # chunk's scores so PE never drains serially at chunk boundaries
            for step in range(KT_PER_B):
                if prev is not None and step < PV_LAG:
                    prev[2](KT_PER_B - PV_LAG + step)
                if prev is not None and step == 14:
                    # deferred well past the prev chunk's last PV (step
                    # PV_LAG-1) so the DVE queue never stalls waiting on it
                    prev_tmp = emit_epi_copy(prev[0], prev[1])
                    emit_epi_rest(prev[0], prev[1], prev_tmp)
                if step in (3, 5, 7, 9):
                    # lepe taps spread over 4 steps so the ~20-matmul bursts
                    # don't delay the score stream (which paces the exps)
                    qs = (step - 3) // 2
                    if qs == 0:
                        # full-height zeroing matmul: clears the whole pvb
                        # bank's has_written bits across ALL 128 partitions
                        # (the 64-row lepe taps alone would leave partitions
                        # 64-127 stale)
                        nc.tensor.matmul(pvb[:, 196:196 + HD],
                                         zrow_sb[0:1, 0:128], one_sb[0:1, 0:HD],
                                         start=True, stop=False,
                                         tile_position=(0, 0),
                                         skip_group_check=True)
                    emit_lepe_taps(qs)
                kt = step
                bidx = kt // EXP_BW
                if kt % EXP_BW == 0:
                    st_of_batch[bidx] = stp.tile([128, 512 * EXP_BW],
                                                 dt.float32, name="st", tag="st")
                kc = bc * 8 + kt // 4
                koff = (kt % 4) * 128
                j = kt % EXP_BW
                # fp8 DoubleRow: two 24-feature half-contractions fused
                nc.tensor.matmul(st_of_batch[bidx][:, 512 * j:512 * j + 512],
                                 qkdr4[0:24, kc, 2:4, koff:koff + 128],
                                 qkdr4[0:24, cc, 0:2, :],
                                 perf_mode=PM.DoubleRow)
                if bi_box[0] < len(batches) and kt + 1 == batches[bi_box[0]][1]:
                    emit_exp(bi_box[0])
                    bi_box[0] += 1
                pvkt = step - PV_LAG
                if pvkt >= 0:
                    emit_pv(pvkt)

            prev = (cc, pvb, emit_pv)

        # drain the last chunk's PV tail + epilogue
        for i in range(PV_LAG):
            prev[2](KT_PER_B - PV_LAG + i)
        emit_epi_rest(prev[0], prev[1], emit_epi_copy(prev[0], prev[1]))
        if _CACHE.get("debug"):
            nc.sync.dma_start(dbg_va_d, v_aug[:])
            nc.sync.dma_start(dbg_vp_d, vT_pad[:])

    nc.compile()
    return nc


def _prep_in_maps(x, qkv_w, qkv_b, lepe_w, lepe_b):
    bf16 = ml_dtypes.bfloat16
    X = np.asarray(x, dtype=np.float32).reshape(SEQ, DIM)
    xT = np.ascontiguousarray(X.T).astype(bf16).reshape(3, 128, SEQ)

    qkv_w = np.asarray(qkv_w, dtype=np.float32)
    qkv_b = np.asarray(qkv_b, dtype=np.float32)
    lepe_w = np.asarray(lepe_w, dtype=np.float32)
    lepe_b = np.asarray(lepe_b, dtype=np.float32)

    idn = np.zeros((128, 64), dtype=np.float32)
    idn[0:64, 0:64] = np.eye(64, dtype=np.float32)
    idn[64:128, 0:64] = np.eye(64, dtype=np.float32)

    in_maps = []
    for h in range(NUM_HEADS):
        sl = slice(h * HD, (h + 1) * HD)
        wq = qkv_w[sl, :] * QS                       # [48, 384], exp-scaled
        wk = qkv_w[DIM + h * HD:DIM + (h + 1) * HD, :]
        wv = qkv_w[2 * DIM + h * HD:2 * DIM + (h + 1) * HD, :]
        wqk = np.zeros((3, 128, 96), dtype=np.float32)
        for c in range(3):
            wqk[c, :, 0:HD] = wq.T[c * 128:(c + 1) * 128]
            wqk[c, :, HD:96] = wk.T[c * 128:(c + 1) * 128]
        wvc = np.ascontiguousarray(wv.T).reshape(3, 128, HD)

        bqk = np.zeros((128, 1), dtype=np.float32)
        bqk[0:HD, 0] = qkv_b[sl] * QS
        bqk[HD:96, 0] = qkv_b[DIM + h * HD:DIM + (h + 1) * HD]

        bv = np.zeros((128, 1), dtype=np.float32)
        bv[0:HD, 0] = qkv_b[2 * DIM + h * HD:2 * DIM + (h + 1) * HD]
        bv[64:64 + HD, 0] = qkv_b[2 * DIM + h * HD:2 * DIM + (h + 1) * HD]

        lw = lepe_w[sl, 0].reshape(HD, 9)            # [48, 9] taps row-major
        dwt = np.zeros((9, 128, HD), dtype=np.float32)
        for i in range(9):
            dwt[i, 0:HD, :] = np.diag(lw[:, i])
            dwt[i, 64:64 + HD, :] = np.diag(lw[:, i])
        lb48 = np.zeros((128, HD), dtype=np.float32)
        lb48[0, :] = lepe_b[sl]
        blob = np.zeros((128, 1024), dtype=bf16)
        for c in range(3):
            blob[:, 96 * c:96 * c + 96] = wqk[c].astype(bf16)
            blob[:, 288 + HD * c:288 + HD * c + HD] = wvc[c].astype(bf16)
        for i in range(9):
            blob[:, 432 + HD * i:432 + HD * i + HD] = dwt[i].astype(bf16)
        blob[:, 864:912] = lb48.astype(bf16)
        blob[:, 912:976] = idn.astype(bf16)
        scal = np.concatenate(
            [bqk, bv, np.full((128, 1), -S0, np.float32)], axis=1
        ).astype(np.float32)
        blob[:, 976:982] = scal.view(np.uint16).view(bf16)

        in_maps.append({
            "xT": xT,
            "blob": blob,
        })
    return in_maps


def kernel(x, qkv_w, qkv_b, lepe_w, lepe_b, H=64, W=64):
    assert int(H) == 64 and int(W) == 64
    from concourse.bass_utils import run_bass_kernel_spmd

    if "nc" not in _CACHE:
        _CACHE["nc"] = _build_module()
    nc = _CACHE["nc"]

    in_maps = _prep_in_maps(x, qkv_w, qkv_b, lepe_w, lepe_b)
    res = run_bass_kernel_spmd(nc, in_maps, core_ids=list(range(NUM_HEADS)))

    full = np.empty((SEQ, DIM), dtype=np.float32)
    for h in range(NUM_HEADS):
        full[:, h * HD:(h + 1) * HD] = res.results[h]["out"].reshape(SEQ, HD)
    return full.reshape(B, N, DIM)
